# revision 11
# baseline (speedup 1.0000x reference)
"""PeakLocalMax (41x41 NMS mask) Trainium2 Bass kernel.

Input : batch_heatmap (16, 1024, 1024, 2) float32
Output: bool mask, same shape: (x == maxpool41x41(x)) & (x > 0.5)

Strategy (per core; batch sharded 2 images/core over 8 cores):
  - Exact f32 separable sliding-window max via van Herk/Gil-Werman:
    segmented prefix/suffix max scans implemented with tensor_tensor_scan
    (op0=min with a block-reset mask, op1=max) + a fused 3-way max
    (scalar_tensor_tensor) combine that also folds in the 0.5 threshold
    as c = nextafter(0.5): M2 = max(window_max, c).
    Un-padded block grid; edge-window clipping comes from persistent
    constant-c margins on the scan-output tiles, so each 1D pass is
    exactly 3 DVE instructions (fwd scan, reversed scan, STT combine).
  - W-direction pass on (H=partitions, W*C=free) tiles using stride-2
    channel views; H-direction pass on PE-transposed strips
    (W=partitions, H=free); transpose back via PE.
  - Final compare offloaded from a full-rate DVE tensor_tensor to a
    half-rate tensor_scalar: the PE computes e = M2 - x in PSUM by
    first writing -x (negated-identity matmul, strided channel rhs)
    and then accumulating the M2^T transpose-back on top (start=False,
    explicit dep edges force accumulation order); the Act engine copies
    e to SBUF; DVE then does out = (e <= 0) as a tensor_scalar, which
    runs in 2x_2p mode (0.52 ns/elem vs 1.04).  fl(M2-x) is sign-exact
    in IEEE fp32, so the mask is exact, ties included.
"""

import os
import sys
import numpy as np

_TRN_REPO = "/opt/trn_rl_repo"

H = 1024
W = 1024
C = 2
B_PER_CORE = 2
N_CORES = 8
V = 20            # min_distance
WIN = 2 * V + 1   # 41
HB = H // 128     # 8 h-blocks
WB = W // 128     # 8 w-blocks
BIG = float(np.float32(3e38))
C05 = float(np.nextafter(np.float32(0.5), np.float32(1)))

_CACHE = {}


def _build():
    if _TRN_REPO not in sys.path:
        sys.path.insert(0, _TRN_REPO)
    from contextlib import ExitStack
    from concourse import bacc, mybir
    import concourse.tile as tile
    from concourse.masks import make_identity
    from concourse.bass import _add_dep_helper

    F32 = mybir.dt.float32
    U8 = mybir.dt.uint8
    Alu = mybir.AluOpType

    nc = bacc.Bacc("TRN2", debug=False, num_devices=N_CORES)
    x_d = nc.dram_tensor("x", [B_PER_CORE, H, W * C], F32, kind="ExternalInput").ap()
    y_d = nc.dram_tensor("y", [B_PER_CORE, H, W * C], U8, kind="ExternalOutput").ap()

    with tile.TileContext(nc) as tc, ExitStack() as ctx:
        sb = ctx.enter_context(tc.tile_pool(name="sb", bufs=1))
        xpool = ctx.enter_context(tc.tile_pool(name="xp", bufs=1))
        spool = ctx.enter_context(tc.tile_pool(name="sp", bufs=1))
        ps = ctx.enter_context(tc.tile_pool(name="ps", bufs=1, space="PSUM"))

        # start the first two x row-tile loads immediately so the setup
        # constants (memsets/iota) overlap the DMA latency
        pre_xt = []
        for hb in range(2):
            t = xpool.tile([128, W * C], F32, name=f"xt0_{hb}", tag="xt", bufs=2)
            nc.sync.dma_start(t[:], x_d[0, hb * 128:(hb + 1) * 128])
            pre_xt.append(t)

        # constants: scan reset masks + PE identity (and negated identity)
        # fwd: reset at k % 41 == 0 ; rev (suffix, scanned backwards):
        # reset at k % 41 == 40 plus the truncated tail element 1023.
        mf = sb.tile([128, W], F32, name="mf")
        mr = sb.tile([128, W], F32, name="mr")
        idn = sb.tile([128, 128], F32, name="idn")
        nidn = sb.tile([128, 128], F32, name="nidn")
        nc.vector.memset(mf[:], BIG)
        nc.vector.memset(mf[:, 0:W:WIN], -BIG)
        nc.vector.memset(mr[:], BIG)
        nc.vector.memset(mr[:, V * 2:W:WIN], -BIG)
        nc.vector.memset(mr[:, W - 1:W], -BIG)
        make_identity(nc, idn[:])
        nc.vector.tensor_scalar(nidn[:], idn[:], -1.0, None, op0=Alu.mult)

        # Persistent scan-output tiles with constant-c margins so each
        # combine is a single full-width STT (no edge-clip ops):
        #   Se_ext = [ c*20 | suffix-scan(1024) ]  -> Se_ext[w] = S[w-20] or c
        #   Pe_ext = [ prefix-scan(1024) | c*20 ]  -> Pe_ext[w+20] = P[w+20] or c
        # Two of each (manual double-buffer via unit-counter parity).
        EXT = W + V
        cnt = [0]
        scan_bufs = {}
        for nm in ("Pw", "Sw", "Ph", "Sh"):
            pair = []
            for i in range(2):
                t = sb.tile([128, EXT], F32, name=f"{nm}{i}")
                if nm[0] == "P":
                    nc.vector.memset(t[:, W:EXT], C05)
                else:
                    nc.vector.memset(t[:, 0:V], C05)
                pair.append(t)
            scan_bufs[nm] = pair

        # segmented scans + combine: out[w] = max(S[w-20], P[w+20], c),
        # window clipping supplied by the constant margins.
        def vh_pass(out_ap, data_ap, u, axis):
            e = nc.vector
            Pe = scan_bufs["Pw" if axis == "w" else "Ph"][u % 2]
            Se = scan_bufs["Sw" if axis == "w" else "Sh"][u % 2]
            e.tensor_tensor_scan(Pe[:, 0:W], mf[:], data_ap,
                                 -BIG, op0=Alu.min, op1=Alu.max)
            e.tensor_tensor_scan(Se[:, V:EXT][:, ::-1], mr[:, ::-1],
                                 data_ap[:, ::-1],
                                 -BIG, op0=Alu.min, op1=Alu.max)
            e.scalar_tensor_tensor(out_ap, Se[:, 0:W], C05, Pe[:, V:EXT],
                                   op0=Alu.max, op1=Alu.max)

        # persistent strips: one [128, W] tile per (ch, wb), reused in place
        # across images; explicit column-granular WAR edges order img k+1's
        # writes against img k's transpose-back reads.
        strips = {}
        for ch in range(C):
            for wb in range(WB):
                strips[(ch, wb)] = sb.tile([128, W], F32, name=f"st_{ch}_{wb}")
        strip_last = {}

        # paced x reload queue: xc tiles feed the compare phases; emitting a
        # load allocates a FIFO slot (bufs=5), so loads are pumped at most 5
        # ahead of their consumption to avoid blocking the DMA queue.
        xcs = {0: {}, 1: {}}
        _xc_pending = [(i, h) for i in range(B_PER_CORE) for h in range(HB)]

        def pump_xc():
            if not _xc_pending:
                return
            img, hb = _xc_pending.pop(0)
            xc = sb.tile([128, W * C], F32, name=f"xc{img}_{hb}",
                         tag="xc", bufs=6)
            nc.sync.dma_start(xc[:], x_d[img, hb * 128:(hb + 1) * 128])
            xcs[img][hb] = xc

        def w_unit(img, hb, xt=None):
            """Load x row-tile, run W-direction van Herk, transpose into strips."""
            if xt is None:
                xt = xpool.tile([128, W * C], F32, name=f"xt{img}_{hb}",
                                tag="xt", bufs=2)
                nc.sync.dma_start(xt[:], x_d[img, hb * 128:(hb + 1) * 128])
            for ch in range(C):
                xv = xt[:, ch:W * C:2]
                R = sb.tile([128, W], F32, name="R", tag="R", bufs=2)
                cnt[0] += 1
                vh_pass(R[:], xv, cnt[0], "w")
                for wb in range(WB):
                    pt = ps.tile([128, 128], F32, name="pt", tag="pt", bufs=3)
                    nc.tensor.transpose(pt[:], R[:, wb * 128:(wb + 1) * 128],
                                        idn[:])
                    cp = nc.scalar.copy(
                        strips[(ch, wb)][:, hb * 128:(hb + 1) * 128], pt[:])
                    key = (img - 1, ch, wb, hb)
                    if key in strip_last:
                        _add_dep_helper(cp.ins, strip_last[key].ins,
                                        True, "strip column reuse")

        def v_unit(ch, wb):
            st = strips[(ch, wb)]
            cnt[0] += 1
            vh_pass(st[:, 0:W], st[:], cnt[0], "h")

        def cmp_ts(img, hb):
            """Compare via e = M2 - x on PE, Act copy, DVE tensor_scalar.
            DVE-cheap (594ns/ch-unit) but PE-heavy: use only where the PE
            chain hides under other DVE work."""
            xc = xcs[img][hb]
            ot = sb.tile([128, W * C], U8, name="ot", tag="ot", bufs=3)
            for ch in range(C):
                m2p = ps.tile([128, W], F32, name="m2p", tag="m2p", bufs=2)
                mms = []
                for mb in range(2):
                    mm = nc.tensor.matmul(
                        m2p[:, mb * 512:(mb + 1) * 512], nidn[:],
                        xc[:, ch + 2 * mb * 512:ch + 2 * mb * 512 + 1024 - ch:2],
                        start=True, stop=False, skip_group_check=True)
                    mms.append(mm)
                for wb in range(WB):
                    tb = nc.tensor.matmul(
                        m2p[:, wb * 128:(wb + 1) * 128],
                        strips[(ch, wb)][:, hb * 128:(hb + 1) * 128],
                        idn[:], is_transpose=True,
                        start=False, stop=True, skip_group_check=True)
                    _add_dep_helper(tb.ins, mms[wb // 4].ins,
                                    True, "psum accum order")
                    strip_last[(img, ch, wb, hb)] = tb
                es = sb.tile([128, W], F32, name="es", tag="es", bufs=3)
                nc.scalar.copy(es[:], m2p[:])
                nc.vector.tensor_scalar(ot[:, ch:W * C:2], es[:], 0.0, None,
                                        op0=Alu.is_le)
            nc.sync.dma_start(y_d[img, hb * 128:(hb + 1) * 128], ot[:])

        def cmp_tt(img, hb):
            """Baseline compare: transpose-back to PSUM + DVE tensor_tensor.
            DVE-dense (1192ns/ch-unit) -- right for the final tail where no
            other DVE work exists to hide a PE chain under."""
            xc = xcs[img][hb]
            ot = sb.tile([128, W * C], U8, name="ot", tag="ot", bufs=3)
            for ch in range(C):
                m2p = ps.tile([128, W], F32, name="m2p", tag="m2p", bufs=2)
                for wb in range(WB):
                    tb = nc.tensor.transpose(
                        m2p[:, wb * 128:(wb + 1) * 128],
                        strips[(ch, wb)][:, hb * 128:(hb + 1) * 128],
                        idn[:])
                    strip_last[(img, ch, wb, hb)] = tb
                nc.vector.tensor_tensor(
                    ot[:, ch:W * C:2], xc[:, ch:W * C:2], m2p[:], op=Alu.is_ge)
            nc.sync.dma_start(y_d[img, hb * 128:(hb + 1) * 128], ot[:])

        # ---- software-pipelined schedule over the two images ----
        for hb in range(HB):
            w_unit(0, hb, xt=(pre_xt[hb] if hb < len(pre_xt) else None))
            if hb < 5:
                pump_xc()
        for ch in range(C):
            for wb in range(WB):
                v_unit(ch, wb)
        # steady: img0 compare interleaved with img1 W-pass.  cmp before
        # w_unit so the strip-column WAR edge (cp after tb) can be added,
        # and the compare PE/Act chain hides under the W-pass DVE scans.
        for hb in range(HB):
            cmp_ts(0, hb)
            w_unit(1, hb)
            pump_xc()
        # pre-pump the tail's remaining x loads so the DMA is hidden
        # under the img1 V-pass instead of pacing the compare tail
        pump_xc()
        for ch in range(C):
            for wb in range(WB):
                v_unit(ch, wb)
            pump_xc()
        for hb in range(HB):
            cmp_tt(1, hb)
            pump_xc()

    nc.compile()
    return nc


def _get_nc():
    if "nc" not in _CACHE:
        _CACHE["nc"] = _build()
    return _CACHE["nc"]


def _install_neff_cache():
    """Cache compiled NEFFs on disk keyed by BIR hash (compile is ~10 min)."""
    if _CACHE.get("neff_cache"):
        return
    import hashlib
    import shutil
    from concourse import bass_utils, bass2jax

    real = bass_utils.compile_bir_kernel
    cache_dir = "/tmp/bass_neff_cache"

    def cached(bir_json, tmpdir, neff_name="file.neff"):
        os.makedirs(cache_dir, exist_ok=True)
        key = hashlib.sha256(bir_json).hexdigest()[:32]
        hit = os.path.join(cache_dir, key + ".neff")
        dst = os.path.join(tmpdir, neff_name)
        if os.path.exists(hit):
            shutil.copyfile(hit, dst)
            return dst
        out = real(bir_json, tmpdir, neff_name)
        try:
            shutil.copyfile(out, hit)
        except OSError:
            pass
        return out

    bass_utils.compile_bir_kernel = cached
    if getattr(bass2jax, "compile_bir_kernel", None) is not None:
        bass2jax.compile_bir_kernel = cached
    _CACHE["neff_cache"] = True


def kernel(batch_heatmap: np.ndarray) -> np.ndarray:
    if _TRN_REPO not in sys.path:
        sys.path.insert(0, _TRN_REPO)
    from concourse.bass_utils import run_bass_kernel_spmd
    _install_neff_cache()

    x = np.ascontiguousarray(np.asarray(batch_heatmap, dtype=np.float32))
    assert x.shape == (16, H, W, C), x.shape
    nc = _get_nc()
    in_maps = [
        {"x": x[B_PER_CORE * r:B_PER_CORE * (r + 1)].reshape(B_PER_CORE, H, W * C)}
        for r in range(N_CORES)
    ]
    res = run_bass_kernel_spmd(nc, in_maps, list(range(N_CORES)))
    out = np.stack([res.results[r]["y"] for r in range(N_CORES)])
    return out.reshape(16, H, W, C).astype(bool)


# revision 12
# speedup vs baseline: 1.0215x; 1.0215x over previous
"""PeakLocalMax (41x41 NMS mask) Trainium2 Bass kernel.

Input : batch_heatmap (16, 1024, 1024, 2) float32
Output: bool mask, same shape: (x == maxpool41x41(x)) & (x > 0.5)

Strategy (per core; batch sharded 2 images/core over 8 cores):
  - Exact f32 separable sliding-window max via van Herk/Gil-Werman:
    segmented prefix/suffix max scans implemented with tensor_tensor_scan
    (op0=min with a block-reset mask, op1=max) + a fused 3-way max
    (scalar_tensor_tensor) combine that also folds in the 0.5 threshold
    as c = nextafter(0.5): M2 = max(window_max, c).
    Un-padded block grid; edge-window clipping comes from persistent
    constant-c margins on the scan-output tiles, so each 1D pass is
    exactly 3 DVE instructions (fwd scan, reversed scan, STT combine).
  - W-direction pass on (H=partitions, W*C=free) tiles using stride-2
    channel views; H-direction pass on PE-transposed strips
    (W=partitions, H=free); transpose back via PE.
  - Final compare offloaded from a full-rate DVE tensor_tensor to a
    half-rate tensor_scalar: the PE computes e = M2 - x in PSUM by
    first writing -x (negated-identity matmul, strided channel rhs)
    and then accumulating the M2^T transpose-back on top (start=False,
    explicit dep edges force accumulation order); the Act engine copies
    e to SBUF; DVE then does out = (e <= 0) as a tensor_scalar, which
    runs in 2x_2p mode (0.52 ns/elem vs 1.04).  fl(M2-x) is sign-exact
    in IEEE fp32, so the mask is exact, ties included.
"""

import os
import sys
import numpy as np

_TRN_REPO = "/opt/trn_rl_repo"

H = 1024
W = 1024
C = 2
B_PER_CORE = 2
N_CORES = 8
V = 20            # min_distance
WIN = 2 * V + 1   # 41
HB = H // 128     # 8 h-blocks
WB = W // 128     # 8 w-blocks
BIG = float(np.float32(3e38))
C05 = float(np.nextafter(np.float32(0.5), np.float32(1)))

_CACHE = {}


def _build():
    if _TRN_REPO not in sys.path:
        sys.path.insert(0, _TRN_REPO)
    from contextlib import ExitStack
    from concourse import bacc, mybir
    import concourse.tile as tile
    from concourse.masks import make_identity
    from concourse.bass import _add_dep_helper

    F32 = mybir.dt.float32
    U8 = mybir.dt.uint8
    Alu = mybir.AluOpType

    nc = bacc.Bacc("TRN2", debug=False, num_devices=N_CORES)
    x_d = nc.dram_tensor("x", [B_PER_CORE, H, W * C], F32, kind="ExternalInput").ap()
    y_d = nc.dram_tensor("y", [B_PER_CORE, H, W * C], U8, kind="ExternalOutput").ap()

    with tile.TileContext(nc) as tc, ExitStack() as ctx:
        sb = ctx.enter_context(tc.tile_pool(name="sb", bufs=1))
        xpool = ctx.enter_context(tc.tile_pool(name="xp", bufs=1))
        spool = ctx.enter_context(tc.tile_pool(name="sp", bufs=1))
        ps = ctx.enter_context(tc.tile_pool(name="ps", bufs=1, space="PSUM"))

        # constants: scan reset masks + PE identity (and negated identity)
        # fwd: reset at k % 41 == 0 ; rev (suffix, scanned backwards):
        # reset at k % 41 == 40 plus the truncated tail element 1023.
        mf = sb.tile([128, W], F32, name="mf")
        mr = sb.tile([128, W], F32, name="mr")
        idn = sb.tile([128, 128], F32, name="idn")
        nidn = sb.tile([128, 128], F32, name="nidn")
        nc.vector.memset(mf[:], BIG)
        nc.vector.memset(mf[:, 0:W:WIN], -BIG)
        nc.vector.memset(mr[:], BIG)
        nc.vector.memset(mr[:, V * 2:W:WIN], -BIG)
        nc.vector.memset(mr[:, W - 1:W], -BIG)
        make_identity(nc, idn[:])
        nc.vector.tensor_scalar(nidn[:], idn[:], -1.0, None, op0=Alu.mult)

        # Persistent scan-output tiles with constant-c margins so each
        # combine is a single full-width STT (no edge-clip ops):
        #   Se_ext = [ c*20 | suffix-scan(1024) ]  -> Se_ext[w] = S[w-20] or c
        #   Pe_ext = [ prefix-scan(1024) | c*20 ]  -> Pe_ext[w+20] = P[w+20] or c
        # Two of each (manual double-buffer via unit-counter parity).
        EXT = W + V
        cnt = [0]
        scan_bufs = {}
        for nm in ("Pw", "Sw", "Ph", "Sh"):
            pair = []
            for i in range(2):
                t = sb.tile([128, EXT], F32, name=f"{nm}{i}")
                if nm[0] == "P":
                    nc.vector.memset(t[:, W:EXT], C05)
                else:
                    nc.vector.memset(t[:, 0:V], C05)
                pair.append(t)
            scan_bufs[nm] = pair

        # segmented scans + combine: out[w] = max(S[w-20], P[w+20], c),
        # window clipping supplied by the constant margins.
        def vh_pass(out_ap, data_ap, u, axis):
            e = nc.vector
            Pe = scan_bufs["Pw" if axis == "w" else "Ph"][u % 2]
            Se = scan_bufs["Sw" if axis == "w" else "Sh"][u % 2]
            e.tensor_tensor_scan(Pe[:, 0:W], mf[:], data_ap,
                                 -BIG, op0=Alu.min, op1=Alu.max)
            e.tensor_tensor_scan(Se[:, V:EXT][:, ::-1], mr[:, ::-1],
                                 data_ap[:, ::-1],
                                 -BIG, op0=Alu.min, op1=Alu.max)
            e.scalar_tensor_tensor(out_ap, Se[:, 0:W], C05, Pe[:, V:EXT],
                                   op0=Alu.max, op1=Alu.max)

        # persistent strips: one [128, W] tile per (ch, wb), reused in place
        # across images; explicit column-granular WAR edges order img k+1's
        # writes against img k's transpose-back reads.
        strips = {}
        for ch in range(C):
            for wb in range(WB):
                strips[(ch, wb)] = sb.tile([128, W], F32, name=f"st_{ch}_{wb}")
        strip_last = {}

        # paced x reload queue: xc tiles feed the compare phases; emitting a
        # load allocates a FIFO slot (bufs=5), so loads are pumped at most 5
        # ahead of their consumption to avoid blocking the DMA queue.
        xcs = {0: {}, 1: {}}
        _xc_pending = [(i, h) for i in range(B_PER_CORE) for h in range(HB)]

        def pump_xc():
            if not _xc_pending:
                return
            img, hb = _xc_pending.pop(0)
            xc = sb.tile([128, W * C], F32, name=f"xc{img}_{hb}",
                         tag="xc", bufs=5)
            nc.sync.dma_start(xc[:], x_d[img, hb * 128:(hb + 1) * 128])
            xcs[img][hb] = xc

        def w_unit(img, hb, xt=None):
            """Load x row-tile, run W-direction van Herk, transpose into strips."""
            if xt is None:
                xt = xpool.tile([128, W * C], F32, name=f"xt{img}_{hb}",
                                tag="xt", bufs=2)
                nc.sync.dma_start(xt[:], x_d[img, hb * 128:(hb + 1) * 128])
            for ch in range(C):
                xv = xt[:, ch:W * C:2]
                R = sb.tile([128, W], F32, name="R", tag="R", bufs=2)
                cnt[0] += 1
                vh_pass(R[:], xv, cnt[0], "w")
                for wb in range(WB):
                    pt = ps.tile([128, 128], F32, name="pt", tag="pt", bufs=3)
                    nc.tensor.transpose(pt[:], R[:, wb * 128:(wb + 1) * 128],
                                        idn[:])
                    cp = nc.scalar.copy(
                        strips[(ch, wb)][:, hb * 128:(hb + 1) * 128], pt[:])
                    key = (img - 1, ch, wb, hb)
                    if key in strip_last:
                        _add_dep_helper(cp.ins, strip_last[key].ins,
                                        True, "strip column reuse")

        def v_unit(ch, wb):
            st = strips[(ch, wb)]
            cnt[0] += 1
            vh_pass(st[:, 0:W], st[:], cnt[0], "h")

        def cmp_ts(img, hb):
            """Compare via e = M2 - x on PE, Act copy, DVE tensor_scalar.
            DVE-cheap (594ns/ch-unit) but PE-heavy: use only where the PE
            chain hides under other DVE work."""
            xc = xcs[img][hb]
            ot = sb.tile([128, W * C], U8, name="ot", tag="ot", bufs=3)
            for ch in range(C):
                m2p = ps.tile([128, W], F32, name="m2p", tag="m2p", bufs=2)
                mms = []
                for mb in range(2):
                    mm = nc.tensor.matmul(
                        m2p[:, mb * 512:(mb + 1) * 512], nidn[:],
                        xc[:, ch + 2 * mb * 512:ch + 2 * mb * 512 + 1024 - ch:2],
                        start=True, stop=False, skip_group_check=True)
                    mms.append(mm)
                for wb in range(WB):
                    tb = nc.tensor.matmul(
                        m2p[:, wb * 128:(wb + 1) * 128],
                        strips[(ch, wb)][:, hb * 128:(hb + 1) * 128],
                        idn[:], is_transpose=True,
                        start=False, stop=True, skip_group_check=True)
                    _add_dep_helper(tb.ins, mms[wb // 4].ins,
                                    True, "psum accum order")
                    strip_last[(img, ch, wb, hb)] = tb
                es = sb.tile([128, W], F32, name="es", tag="es", bufs=3)
                nc.scalar.copy(es[:], m2p[:])
                nc.vector.tensor_scalar(ot[:, ch:W * C:2], es[:], 0.0, None,
                                        op0=Alu.is_le)
            nc.sync.dma_start(y_d[img, hb * 128:(hb + 1) * 128], ot[:])

        def cmp_tt(img, hb):
            """Baseline compare: transpose-back to PSUM + DVE tensor_tensor.
            DVE-dense (1192ns/ch-unit) -- right for the final tail where no
            other DVE work exists to hide a PE chain under."""
            xc = xcs[img][hb]
            ot = sb.tile([128, W * C], U8, name="ot", tag="ot", bufs=3)
            for ch in range(C):
                m2p = ps.tile([128, W], F32, name="m2p", tag="m2p", bufs=2)
                for wb in range(WB):
                    tb = nc.tensor.transpose(
                        m2p[:, wb * 128:(wb + 1) * 128],
                        strips[(ch, wb)][:, hb * 128:(hb + 1) * 128],
                        idn[:])
                    strip_last[(img, ch, wb, hb)] = tb
                nc.vector.tensor_tensor(
                    ot[:, ch:W * C:2], xc[:, ch:W * C:2], m2p[:], op=Alu.is_ge)
            nc.sync.dma_start(y_d[img, hb * 128:(hb + 1) * 128], ot[:])

        # ---- software-pipelined schedule over the two images ----
        for hb in range(HB):
            w_unit(0, hb)
            if hb < 5:
                pump_xc()
        for ch in range(C):
            for wb in range(WB):
                v_unit(ch, wb)
        # steady: img0 compare interleaved with img1 W-pass.  cmp before
        # w_unit so the strip-column WAR edge (cp after tb) can be added,
        # and the compare PE/Act chain hides under the W-pass DVE scans.
        for hb in range(HB):
            cmp_ts(0, hb)
            w_unit(1, hb)
            pump_xc()
        for ch in range(C):
            for wb in range(WB):
                v_unit(ch, wb)
        for hb in range(HB):
            cmp_tt(1, hb)
            pump_xc()

    nc.compile()
    return nc


def _get_nc():
    if "nc" not in _CACHE:
        _CACHE["nc"] = _build()
    return _CACHE["nc"]


def _install_neff_cache():
    """Cache compiled NEFFs on disk keyed by BIR hash (compile is ~10 min)."""
    if _CACHE.get("neff_cache"):
        return
    import hashlib
    import shutil
    from concourse import bass_utils, bass2jax

    real = bass_utils.compile_bir_kernel
    cache_dir = "/tmp/bass_neff_cache"

    def cached(bir_json, tmpdir, neff_name="file.neff"):
        os.makedirs(cache_dir, exist_ok=True)
        key = hashlib.sha256(bir_json).hexdigest()[:32]
        hit = os.path.join(cache_dir, key + ".neff")
        dst = os.path.join(tmpdir, neff_name)
        if os.path.exists(hit):
            shutil.copyfile(hit, dst)
            return dst
        out = real(bir_json, tmpdir, neff_name)
        try:
            shutil.copyfile(out, hit)
        except OSError:
            pass
        return out

    bass_utils.compile_bir_kernel = cached
    if getattr(bass2jax, "compile_bir_kernel", None) is not None:
        bass2jax.compile_bir_kernel = cached
    _CACHE["neff_cache"] = True


def kernel(batch_heatmap: np.ndarray) -> np.ndarray:
    if _TRN_REPO not in sys.path:
        sys.path.insert(0, _TRN_REPO)
    from concourse.bass_utils import run_bass_kernel_spmd
    _install_neff_cache()

    x = np.ascontiguousarray(np.asarray(batch_heatmap, dtype=np.float32))
    assert x.shape == (16, H, W, C), x.shape
    nc = _get_nc()
    in_maps = [
        {"x": x[B_PER_CORE * r:B_PER_CORE * (r + 1)].reshape(B_PER_CORE, H, W * C)}
        for r in range(N_CORES)
    ]
    res = run_bass_kernel_spmd(nc, in_maps, list(range(N_CORES)))
    out = np.stack([res.results[r]["y"] for r in range(N_CORES)])
    return out.reshape(16, H, W, C).astype(bool)
